# revision 5
# baseline (speedup 1.0000x reference)
"""GroupGRUCell with shared schema-pool parameters — Trainium2 Bass kernel.

Problem shapes (hardcoded): B=256 batch, U=64 GRU units, DIN=H=256, S=8 schemas.
  Wx[u] = sum_s sw_x[u,s] * pool_x[s].T   (per-unit weights from shared pool)
  gate_x = x @ Wx ; gate_h = h @ Wh ; standard GRU cell gate math.

Sharding: unit-parallel across 8 NeuronCores (8 units per core); the schema
pool is replicated per core. Per core:
  - weight combine: mostly on the PE via scaled-identity accumulation —
    matmul(psum, lhsT=c_s*I, rhs=P_s chunk, accumulate over s). The PE reads
    rhs at 128 elem/cycle at 2.4GHz, which beats any vector-engine path, and
    dense PE work keeps the HAM clock-gate at 8/8. A few chains stay on DVE
    (tensor_scalar mul + tensor_tensor add, both in fast perf modes) for
    engine balance. Scaled identities c_s*I are built by tiny DVE TS ops.
  - ACT copies combined weights PSUM->SBUF (bf16) and does sigmoid/tanh.
  - GPSIMD takes the d/e gate ops; DVE the rest of the gate math in bf16.
"""

import numpy as np
import ml_dtypes

B, U, DIN, H, S = 256, 64, 256, 256, 8
NCORES = 8
UC = U // NCORES  # units per core
O3 = 3 * H        # 768
KC = DIN // 128   # 2 contraction chunks
MC = B // 128     # 2 batch chunks

BF16 = ml_dtypes.bfloat16

# chain index = u*2 + (0 for x, 1 for h); chains listed here run on DVE,
# the rest on the PE.
DVE_CHAINS = frozenset({0, 1, 8})


def _build_program():
    from contextlib import ExitStack

    import concourse.bacc as bacc
    import concourse.bass as bass
    import concourse.mybir as mybir
    import concourse.tile as tile

    bf = mybir.dt.bfloat16
    f32 = mybir.dt.float32
    AF = mybir.ActivationFunctionType
    ALU = mybir.AluOpType

    nc = bacc.Bacc("TRN2", target_bir_lowering=False, debug=False)

    xT = nc.dram_tensor("xT", [UC, 128, KC, B], bf, kind="ExternalInput")
    hT = nc.dram_tensor("hT", [UC, 128, KC, B], bf, kind="ExternalInput")
    hbh = nc.dram_tensor("hbh", [UC, MC, 128, H], bf, kind="ExternalInput")
    poolx = nc.dram_tensor("poolx", [S, 128, KC * O3], bf, kind="ExternalInput")
    poolh = nc.dram_tensor("poolh", [S, 128, KC * O3], bf, kind="ExternalInput")
    swx = nc.dram_tensor("swx", [128, UC * S], f32, kind="ExternalInput")
    swh = nc.dram_tensor("swh", [128, UC * S], f32, kind="ExternalInput")
    iden = nc.dram_tensor("iden", [128, 128], bf, kind="ExternalInput")
    hy = nc.dram_tensor("hy", [UC, MC, 128, H], bf, kind="ExternalOutput")

    with tile.TileContext(nc) as tc, ExitStack() as ctx:
        pconst = ctx.enter_context(tc.tile_pool(name="pconst", bufs=1))
        pci = ctx.enter_context(tc.tile_pool(name="pci", bufs=3))
        pwacc = ctx.enter_context(tc.tile_pool(name="pwacc", bufs=2))
        pxin = ctx.enter_context(tc.tile_pool(name="pxin", bufs=3))
        pgtmp = ctx.enter_context(tc.tile_pool(name="pgtmp", bufs=4))
        pout = ctx.enter_context(tc.tile_pool(name="pout", bufs=4))
        ppw = ctx.enter_context(tc.tile_pool(name="ppw", bufs=2, space="PSUM"))
        ppsum = ctx.enter_context(tc.tile_pool(name="ppsum", bufs=1, space="PSUM"))

        # Schema pool + mixing weights + identity resident in SBUF throughout.
        px_sb, ph_sb = [], []
        for s in range(S):
            t = pconst.tile([128, KC * O3], bf, tag=f"poolx{s}")
            nc.sync.dma_start(out=t, in_=poolx[s])
            px_sb.append(t)
        for s in range(S):
            t = pconst.tile([128, KC * O3], bf, tag=f"poolh{s}")
            nc.sync.dma_start(out=t, in_=poolh[s])
            ph_sb.append(t)
        swx_sb = pconst.tile([128, UC * S], f32, tag="swx")
        nc.sync.dma_start(out=swx_sb, in_=swx[:, :])
        swh_sb = pconst.tile([128, UC * S], f32, tag="swh")
        nc.sync.dma_start(out=swh_sb, in_=swh[:, :])
        id_sb = pconst.tile([128, 128], bf, tag="iden")
        nc.sync.dma_start(out=id_sb, in_=iden[:, :])

        # Pre-touch constants on DVE (walrus TS/TT structs carry one sync wait;
        # combine-chain ops spend it on a same-proc wait, so DVE must observe
        # the constant-DMA procs up front).
        def _head(t):
            return t[:, 0, 0:1] if len(t.shape) == 3 else t[:, 0:1]

        consts = px_sb + ph_sb + [swx_sb, swh_sb, id_sb]
        for i, t in enumerate(consts):
            sc = pconst.tile([128, 1], f32, tag=f"scr{i}")
            nc.vector.tensor_copy(out=sc, in_=_head(t))

        # psum chunking of the flat [128, KC*O3] weight tiles
        NCH = 3

        for u in range(UC):
            xt = pxin.tile([128, KC, B], bf, tag="xt")
            nc.sync.dma_start(out=xt, in_=xT[u])
            ht = pxin.tile([128, KC, B], bf, tag="ht")
            nc.sync.dma_start(out=ht, in_=hT[u])

            # --- weight combine: w = sum_s sw[u,s] * pool[s] -> bf16 SBUF ---
            wfin = {}
            for ti, (key, psb, swsb) in enumerate(
                (("x", px_sb, swx_sb), ("h", ph_sb, swh_sb))
            ):
                col = u * S
                ci_idx = 2 * u + ti
                if ci_idx in DVE_CHAINS:
                    wa = pwacc.tile([128, KC * O3], bf, tag=f"w{key}a")
                    wb = pwacc.tile([128, KC * O3], bf, tag=f"w{key}b")
                    nc.vector.tensor_scalar(
                        out=wa, in0=psb[0],
                        scalar1=swsb[:, col : col + 1], scalar2=None,
                        op0=ALU.mult,
                    )
                    cur, nxt = wa, wb
                    for s in range(1, S):
                        tmp = pwacc.tile([128, KC * O3], bf, tag=f"w{key}m")
                        nc.vector.tensor_scalar(
                            out=tmp, in0=psb[s],
                            scalar1=swsb[:, col + s : col + s + 1], scalar2=None,
                            op0=ALU.mult,
                        )
                        nc.vector.tensor_tensor(out=nxt, in0=tmp, in1=cur, op=ALU.add)
                        cur, nxt = nxt, cur
                    wfin[key] = cur
                else:
                    # PE scaled-identity combine, accumulated in PSUM.
                    w = pwacc.tile([128, KC * O3], bf, tag=f"w{key}p")
                    pts = []
                    for c in range(NCH):
                        pw_chunk = ppw.tile([128, 512], f32, tag=f"pw{c}")
                        pts.append(pw_chunk)
                    for s in range(S):
                        ci = pci.tile([128, 128], bf, tag="ci")
                        nc.vector.tensor_scalar(
                            out=ci, in0=id_sb,
                            scalar1=swsb[:, col + s : col + s + 1], scalar2=None,
                            op0=ALU.mult,
                        )
                        for c in range(NCH):
                            nc.tensor.matmul(
                                pts[c], ci, psb[s][:, c * 512 : (c + 1) * 512],
                                start=(s == 0), stop=(s == S - 1),
                            )
                    for c in range(NCH):
                        nc.scalar.activation(
                            out=w[:, c * 512 : (c + 1) * 512], in_=pts[c],
                            func=AF.Copy,
                        )
                    wfin[key] = w
            wx, wh = wfin["x"], wfin["h"]

            for mc in range(MC):
                hbt = pxin.tile([128, H], bf, tag="hbt")
                nc.sync.dma_start(out=hbt, in_=hbh[u, mc])

                p_ri = ppsum.tile([128, 512], f32, tag="ri")
                p_n = ppsum.tile([128, 512], f32, tag="n")
                bs = slice(mc * 128, (mc + 1) * 128)
                for kc in range(KC):
                    lx = xt[:, kc, bs]
                    nc.tensor.matmul(
                        p_ri, lx, wx[:, kc * O3 : kc * O3 + 512],
                        start=(kc == 0), stop=False,
                    )
                    nc.tensor.matmul(
                        p_n[:, 0:H], lx, wx[:, kc * O3 + 512 : (kc + 1) * O3],
                        start=(kc == 0), stop=(kc == 1),
                    )
                for kc in range(KC):
                    lh = ht[:, kc, bs]
                    nc.tensor.matmul(
                        p_ri, lh, wh[:, kc * O3 : kc * O3 + 512],
                        start=False, stop=(kc == 1),
                    )
                    nc.tensor.matmul(
                        p_n[:, H:512], lh, wh[:, kc * O3 + 512 : (kc + 1) * O3],
                        start=(kc == 0), stop=(kc == 1),
                    )

                # --- gate math ---
                sig = pgtmp.tile([128, 512], bf, tag="sig")
                nc.scalar.activation(out=sig, in_=p_ri, func=AF.Sigmoid)
                t1 = pgtmp.tile([128, H], f32, tag="t1")
                nc.vector.tensor_tensor(
                    out=t1, in0=sig[:, 0:H], in1=p_n[:, H:512], op=ALU.mult
                )
                t2 = pgtmp.tile([128, H], f32, tag="t2")
                nc.vector.tensor_tensor(out=t2, in0=t1, in1=p_n[:, 0:H], op=ALU.add)
                ng = pgtmp.tile([128, H], bf, tag="ng")
                nc.scalar.activation(out=ng, in_=t2, func=AF.Tanh)
                d = pgtmp.tile([128, H], bf, tag="d")
                nc.gpsimd.tensor_tensor(out=d, in0=hbt, in1=ng, op=ALU.subtract)
                e = pgtmp.tile([128, H], bf, tag="e")
                nc.gpsimd.tensor_tensor(
                    out=e, in0=sig[:, H:512], in1=d, op=ALU.mult
                )
                o = pout.tile([128, H], bf, tag="o")
                nc.vector.tensor_tensor(out=o, in0=ng, in1=e, op=ALU.add)
                nc.sync.dma_start(out=hy[u, mc], in_=o)

    nc.compile()
    return nc


def _prep_inputs(x, hidden, pool_x, pool_h, sw_x, sw_h):
    """Host-side (free) slicing / transposition / casting per core."""
    # pool[s, o, d] -> [s, d, o] -> [s, dp, kc, o]  (d = kc*128 + dp)
    def prep_pool(p):
        pt = np.ascontiguousarray(p.transpose(0, 2, 1))  # [S, DIN, O3]
        pt = pt.reshape(S, KC, 128, O3).transpose(0, 2, 1, 3)  # [s, dp, kc, o]
        pt = pt.reshape(S, 128, KC * O3)
        return np.ascontiguousarray(pt.astype(BF16))

    poolx_h = prep_pool(pool_x)
    poolh_h = prep_pool(pool_h)
    iden_h = np.eye(128, dtype=np.float32).astype(BF16)

    in_maps = []
    for c in range(NCORES):
        us = slice(c * UC, (c + 1) * UC)
        xc = x[:, us, :]  # [B, UC, DIN]
        hc = hidden[:, us, :]
        xT_h = np.ascontiguousarray(
            xc.transpose(1, 2, 0).reshape(UC, KC, 128, B).transpose(0, 2, 1, 3).astype(BF16)
        )
        hT_h = np.ascontiguousarray(
            hc.transpose(1, 2, 0).reshape(UC, KC, 128, B).transpose(0, 2, 1, 3).astype(BF16)
        )
        hbh_h = np.ascontiguousarray(
            hc.transpose(1, 0, 2).reshape(UC, MC, 128, H).astype(BF16)
        )
        swx_h = np.ascontiguousarray(
            np.broadcast_to(
                sw_x[us].reshape(1, UC * S).astype(np.float32), (128, UC * S)
            )
        )
        swh_h = np.ascontiguousarray(
            np.broadcast_to(
                sw_h[us].reshape(1, UC * S).astype(np.float32), (128, UC * S)
            )
        )
        in_maps.append(
            {
                "xT": xT_h,
                "hT": hT_h,
                "hbh": hbh_h,
                "poolx": poolx_h,
                "poolh": poolh_h,
                "swx": swx_h,
                "swh": swh_h,
                "iden": iden_h,
            }
        )
    return in_maps


_CACHED_NC = None


def _get_nc():
    global _CACHED_NC
    if _CACHED_NC is None:
        _CACHED_NC = _build_program()
    return _CACHED_NC


def kernel(x, hidden, pool_x, pool_h, sw_x, sw_h, _trace=False, _results_holder=None):
    from concourse.bass_utils import run_bass_kernel_spmd

    x = np.asarray(x)
    hidden = np.asarray(hidden)
    pool_x = np.asarray(pool_x)
    pool_h = np.asarray(pool_h)
    sw_x = np.asarray(sw_x)
    sw_h = np.asarray(sw_h)

    nc = _get_nc()
    in_maps = _prep_inputs(x, hidden, pool_x, pool_h, sw_x, sw_h)
    res = run_bass_kernel_spmd(
        nc, in_maps, core_ids=list(range(NCORES)), trace=_trace
    )
    if _results_holder is not None:
        _results_holder.append(res)

    out = np.empty((B, U, H), dtype=np.float32)
    for c in range(NCORES):
        hy_c = np.asarray(res.results[c]["hy"]).astype(np.float32)  # [UC, MC, 128, H]
        out[:, c * UC : (c + 1) * UC, :] = hy_c.reshape(UC, B, H).transpose(1, 0, 2)
    return out


# revision 9
# speedup vs baseline: 1.1956x; 1.1956x over previous
"""GroupGRUCell with shared schema-pool parameters — Trainium2 Bass kernel.

Problem shapes (hardcoded): B=256 batch, U=64 GRU units, DIN=H=256, S=8 schemas.
  Wx[u] = sum_s sw_x[u,s] * pool_x[s].T   (per-unit weights from shared pool)
  gate_x = x @ Wx ; gate_h = h @ Wh ; standard GRU cell gate math.

Sharding: unit-parallel across 8 NeuronCores (8 units per core); the schema
pool is replicated per core. Per core:
  - weight combine: mostly on the PE via scaled-identity accumulation —
    matmul(psum, lhsT=c_s*I, rhs=P_s chunk, accumulate over s at 1 col/cycle,
    2.4GHz when the HAM clock-gate is warm). A few chains stay on the vector
    engines for balance: pure-DVE (TS mul + TT add, fast perf modes) and
    ACT-mul+DVE-add variants.
  - scaled identities c_s*I are built up-front (split DVE/ACT) so the PE
    never starves; small constants are DMA'd before the big pool tensors.
  - ACT copies combined weights PSUM->SBUF (bf16) and does sigmoid/tanh.
  - GPSIMD takes the d/e gate ops; DVE the rest of the gate math in bf16.
"""

import numpy as np
import ml_dtypes

B, U, DIN, H, S = 256, 64, 256, 256, 8
NCORES = 8
UC = U // NCORES  # units per core
O3 = 3 * H        # 768
KC = DIN // 128   # 2 contraction chunks
MC = B // 128     # 2 batch chunks
FDW = KC * O3     # 1536 flat weight free-dim

BF16 = ml_dtypes.bfloat16

# chain index = u*2 + (0 for x, 1 for h)
DVE_PURE = frozenset({0, 1, 2})   # u0x, u0h, u1x: TS+TT fully on DVE
ACT_MUL = frozenset({3, 4})       # u1h, u2x: ACT muls + DVE adds


def _build_program():
    from contextlib import ExitStack

    import concourse.bacc as bacc
    import concourse.bass as bass
    import concourse.mybir as mybir
    import concourse.tile as tile

    bf = mybir.dt.bfloat16
    f32 = mybir.dt.float32
    AF = mybir.ActivationFunctionType
    ALU = mybir.AluOpType

    nc = bacc.Bacc("TRN2", target_bir_lowering=False, debug=False)

    xT = nc.dram_tensor("xT", [UC, 128, KC, B], bf, kind="ExternalInput")
    hT = nc.dram_tensor("hT", [UC, 128, KC, B], bf, kind="ExternalInput")
    hbh = nc.dram_tensor("hbh", [UC, MC, 128, H], bf, kind="ExternalInput")
    poolx = nc.dram_tensor("poolx", [S, 128, FDW], bf, kind="ExternalInput")
    poolh = nc.dram_tensor("poolh", [S, 128, FDW], bf, kind="ExternalInput")
    swx = nc.dram_tensor("swx", [128, UC * S], f32, kind="ExternalInput")
    swh = nc.dram_tensor("swh", [128, UC * S], f32, kind="ExternalInput")
    iden = nc.dram_tensor("iden", [128, 128], bf, kind="ExternalInput")
    hy = nc.dram_tensor("hy", [UC, MC, 128, H], bf, kind="ExternalOutput")

    with tile.TileContext(nc) as tc, ExitStack() as ctx:
        pconst = ctx.enter_context(tc.tile_pool(name="pconst", bufs=1))
        pci = ctx.enter_context(tc.tile_pool(name="pci", bufs=1))
        pwx = ctx.enter_context(tc.tile_pool(name="pwx", bufs=1))
        pwacc = ctx.enter_context(tc.tile_pool(name="pwacc", bufs=2))
        pxin = ctx.enter_context(tc.tile_pool(name="pxin", bufs=1))
        phb = ctx.enter_context(tc.tile_pool(name="phb", bufs=4))
        pgtmp = ctx.enter_context(tc.tile_pool(name="pgtmp", bufs=4))
        pout = ctx.enter_context(tc.tile_pool(name="pout", bufs=4))
        ppw = ctx.enter_context(tc.tile_pool(name="ppw", bufs=1, space="PSUM"))
        ppsum = ctx.enter_context(tc.tile_pool(name="ppsum", bufs=2, space="PSUM"))

        # --- small constants first so nothing downstream waits on them ---
        swx_sb = pconst.tile([128, UC * S], f32, tag="swx")
        nc.sync.dma_start(out=swx_sb, in_=swx[:, :])
        swh_sb = pconst.tile([128, UC * S], f32, tag="swh")
        nc.sync.dma_start(out=swh_sb, in_=swh[:, :])
        id_sb = pconst.tile([128, 128], bf, tag="iden")
        nc.sync.dma_start(out=id_sb, in_=iden[:, :])

        px_sb, ph_sb = [], []
        for s in range(S):
            t = pconst.tile([128, FDW], bf, tag=f"poolx{s}")
            nc.sync.dma_start(out=t, in_=poolx[s])
            px_sb.append(t)
        for s in range(S):
            t = pconst.tile([128, FDW], bf, tag=f"poolh{s}")
            nc.sync.dma_start(out=t, in_=poolh[s])
            ph_sb.append(t)

        # inputs resident for the whole kernel (bufs=8)
        xts, hts = [], []
        for u in range(UC):
            xt = pxin.tile([128, KC, B], bf, tag=f"xt{u}")
            nc.sync.dma_start(out=xt, in_=xT[u])
            xts.append(xt)
            ht = pxin.tile([128, KC, B], bf, tag=f"ht{u}")
            nc.sync.dma_start(out=ht, in_=hT[u])
            hts.append(ht)

        def _head(t):
            return t[:, 0, 0:1] if len(t.shape) == 3 else t[:, 0:1]

        # DVE/ACT observe the small consts (one-sync-wait rule for TS chains)
        for i, t in enumerate([swx_sb, swh_sb, id_sb]):
            sc = pconst.tile([128, 1], f32, tag=f"scrv{i}")
            nc.vector.tensor_copy(out=sc, in_=_head(t))
        for i, t in enumerate([swx_sb, swh_sb, id_sb]):
            sc = pconst.tile([128, 1], f32, tag=f"scra{i}")
            nc.scalar.activation(out=sc, in_=_head(t), func=AF.Copy)

        # --- scaled identities for every PE chain, built up front ---
        # ci_tiles[(ci_idx)][s] = sw[u, s] * I  (bf16), split DVE/ACT
        pe_chains = [
            ci for ci in range(2 * UC) if ci not in DVE_PURE and ci not in ACT_MUL
        ]
        ci_tiles = {}
        ci_ct = 0
        for ci_idx in pe_chains:
            u, ti = divmod(ci_idx, 2)
            swsb = swx_sb if ti == 0 else swh_sb
            col = u * S
            tl = []
            for s in range(S):
                ci = pci.tile([128, 128], bf, tag=f"ci{ci_idx}_{s}")
                if ci_ct % 2 == 0:
                    nc.vector.tensor_scalar(
                        out=ci, in0=id_sb,
                        scalar1=swsb[:, col + s : col + s + 1], scalar2=None,
                        op0=ALU.mult,
                    )
                else:
                    nc.scalar.activation(
                        out=ci, in_=id_sb, func=AF.Copy,
                        scale=swsb[:, col + s : col + s + 1],
                    )
                ci_ct += 1
                tl.append(ci)
            ci_tiles[ci_idx] = tl

        # DVE/ACT observe pool_x before the x-side combine chains
        for i, t in enumerate(px_sb):
            sc = pconst.tile([128, 1], f32, tag=f"scrpx{i}")
            nc.vector.tensor_copy(out=sc, in_=_head(t))
        for i, t in enumerate(px_sb):
            sc = pconst.tile([128, 1], f32, tag=f"scrpxa{i}")
            nc.scalar.activation(out=sc, in_=_head(t), func=AF.Copy)

        NCH = 3  # 512-wide psum chunks of the flat 1536 weight tile

        def combine_chain(ci_idx, key, psb, swsb, u):
            """Emit one weight-combine chain; returns the finished W tile.

            x-side results live until phase B consumes them, so they get
            per-unit tags in a bufs=1 pool; h-side tiles rotate (bufs=2).
            """
            col = u * S
            wpool = pwx if key == "x" else pwacc
            wtag = f"w{key}{u}" if key == "x" else f"w{key}"
            if ci_idx in DVE_PURE or ci_idx in ACT_MUL:
                wa = wpool.tile([128, FDW], bf, tag=f"{wtag}a")
                wb = wpool.tile([128, FDW], bf, tag=f"{wtag}b")
                act_mul = ci_idx in ACT_MUL
                if act_mul:
                    nc.scalar.activation(
                        out=wa, in_=psb[0], func=AF.Copy,
                        scale=swsb[:, col : col + 1],
                    )
                else:
                    nc.vector.tensor_scalar(
                        out=wa, in0=psb[0],
                        scalar1=swsb[:, col : col + 1], scalar2=None,
                        op0=ALU.mult,
                    )
                cur, nxt = wa, wb
                for s in range(1, S):
                    tmp = pwacc.tile([128, FDW], bf, tag=f"w{key}m")
                    # tmp rotates; wa/wb hold the running sum
                    if act_mul:
                        nc.scalar.activation(
                            out=tmp, in_=psb[s], func=AF.Copy,
                            scale=swsb[:, col + s : col + s + 1],
                        )
                    else:
                        nc.vector.tensor_scalar(
                            out=tmp, in0=psb[s],
                            scalar1=swsb[:, col + s : col + s + 1], scalar2=None,
                            op0=ALU.mult,
                        )
                    nc.vector.tensor_tensor(out=nxt, in0=tmp, in1=cur, op=ALU.add)
                    cur, nxt = nxt, cur
                return cur
            # PE scaled-identity combine, accumulated in PSUM
            w = wpool.tile([128, FDW], bf, tag=f"{wtag}p")
            pts = []
            for c in range(NCH):
                pw_chunk = ppw.tile([128, 512], f32, tag=f"pw{c}")
                pts.append(pw_chunk)
            cis = ci_tiles[ci_idx]
            for s in range(S):
                for c in range(NCH):
                    nc.tensor.matmul(
                        pts[c], cis[s], psb[s][:, c * 512 : (c + 1) * 512],
                        start=(s == 0), stop=(s == S - 1),
                    )
            for c in range(NCH):
                nc.scalar.activation(
                    out=w[:, c * 512 : (c + 1) * 512], in_=pts[c], func=AF.Copy
                )
            return w

        # --- phase A: all x-side chains ---
        wxs = {}
        for u in range(UC):
            wxs[u] = combine_chain(2 * u, "x", px_sb, swx_sb, u)

        # DVE/ACT observe pool_h before the h-side chains
        for i, t in enumerate(ph_sb):
            sc = pconst.tile([128, 1], f32, tag=f"scrph{i}")
            nc.vector.tensor_copy(out=sc, in_=_head(t))
        for i, t in enumerate(ph_sb):
            sc = pconst.tile([128, 1], f32, tag=f"scrpha{i}")
            nc.scalar.activation(out=sc, in_=_head(t), func=AF.Copy)

        # --- phase B: per unit: h-chain, then matmuls + gate math ---
        for u in range(UC):
            wx = wxs[u]
            wh = combine_chain(2 * u + 1, "h", ph_sb, swh_sb, u)
            xt, ht = xts[u], hts[u]

            for mc in range(MC):
                hbt = phb.tile([128, H], bf, tag="hbt")
                nc.sync.dma_start(out=hbt, in_=hbh[u, mc])

                p_ri = ppsum.tile([128, 512], f32, tag="ri")
                p_n = ppsum.tile([128, 512], f32, tag="n")
                bs = slice(mc * 128, (mc + 1) * 128)
                for kc in range(KC):
                    lx = xt[:, kc, bs]
                    nc.tensor.matmul(
                        p_ri, lx, wx[:, kc * O3 : kc * O3 + 512],
                        start=(kc == 0), stop=False,
                    )
                    nc.tensor.matmul(
                        p_n[:, 0:H], lx, wx[:, kc * O3 + 512 : (kc + 1) * O3],
                        start=(kc == 0), stop=(kc == 1),
                    )
                for kc in range(KC):
                    lh = ht[:, kc, bs]
                    nc.tensor.matmul(
                        p_ri, lh, wh[:, kc * O3 : kc * O3 + 512],
                        start=False, stop=(kc == 1),
                    )
                    nc.tensor.matmul(
                        p_n[:, H:512], lh, wh[:, kc * O3 + 512 : (kc + 1) * O3],
                        start=(kc == 0), stop=(kc == 1),
                    )

                # --- gate math ---
                sig = pgtmp.tile([128, 512], bf, tag="sig")
                nc.scalar.activation(out=sig, in_=p_ri, func=AF.Sigmoid)
                t1 = pgtmp.tile([128, H], f32, tag="t1")
                nc.vector.tensor_tensor(
                    out=t1, in0=sig[:, 0:H], in1=p_n[:, H:512], op=ALU.mult
                )
                t2 = pgtmp.tile([128, H], f32, tag="t2")
                nc.vector.tensor_tensor(
                    out=t2, in0=t1, in1=p_n[:, 0:H], op=ALU.add
                )
                ng = pgtmp.tile([128, H], bf, tag="ng")
                nc.scalar.activation(out=ng, in_=t2, func=AF.Tanh)
                d = pgtmp.tile([128, H], bf, tag="d")
                nc.gpsimd.tensor_tensor(out=d, in0=hbt, in1=ng, op=ALU.subtract)
                e = pgtmp.tile([128, H], bf, tag="e")
                nc.gpsimd.tensor_tensor(
                    out=e, in0=sig[:, H:512], in1=d, op=ALU.mult
                )
                o = pout.tile([128, H], bf, tag="o")
                nc.vector.tensor_tensor(out=o, in0=ng, in1=e, op=ALU.add)
                nc.sync.dma_start(out=hy[u, mc], in_=o)

    nc.compile()
    return nc


def _prep_inputs(x, hidden, pool_x, pool_h, sw_x, sw_h):
    """Host-side (free) slicing / transposition / casting per core."""
    # pool[s, o, d] -> [s, d, o] -> [s, dp, kc*o]  (d = kc*128 + dp)
    def prep_pool(p):
        pt = np.ascontiguousarray(p.transpose(0, 2, 1))  # [S, DIN, O3]
        pt = pt.reshape(S, KC, 128, O3).transpose(0, 2, 1, 3)  # [s, dp, kc, o]
        pt = pt.reshape(S, 128, FDW)
        return np.ascontiguousarray(pt.astype(BF16))

    poolx_h = prep_pool(pool_x)
    poolh_h = prep_pool(pool_h)
    iden_h = np.eye(128, dtype=np.float32).astype(BF16)

    in_maps = []
    for c in range(NCORES):
        us = slice(c * UC, (c + 1) * UC)
        xc = x[:, us, :]  # [B, UC, DIN]
        hc = hidden[:, us, :]
        xT_h = np.ascontiguousarray(
            xc.transpose(1, 2, 0).reshape(UC, KC, 128, B).transpose(0, 2, 1, 3).astype(BF16)
        )
        hT_h = np.ascontiguousarray(
            hc.transpose(1, 2, 0).reshape(UC, KC, 128, B).transpose(0, 2, 1, 3).astype(BF16)
        )
        hbh_h = np.ascontiguousarray(
            hc.transpose(1, 0, 2).reshape(UC, MC, 128, H).astype(BF16)
        )
        swx_h = np.ascontiguousarray(
            np.broadcast_to(
                sw_x[us].reshape(1, UC * S).astype(np.float32), (128, UC * S)
            )
        )
        swh_h = np.ascontiguousarray(
            np.broadcast_to(
                sw_h[us].reshape(1, UC * S).astype(np.float32), (128, UC * S)
            )
        )
        in_maps.append(
            {
                "xT": xT_h,
                "hT": hT_h,
                "hbh": hbh_h,
                "poolx": poolx_h,
                "poolh": poolh_h,
                "swx": swx_h,
                "swh": swh_h,
                "iden": iden_h,
            }
        )
    return in_maps


_CACHED_NC = None


def _get_nc():
    global _CACHED_NC
    if _CACHED_NC is None:
        _CACHED_NC = _build_program()
    return _CACHED_NC


def kernel(x, hidden, pool_x, pool_h, sw_x, sw_h, _trace=False, _results_holder=None):
    from concourse.bass_utils import run_bass_kernel_spmd

    x = np.asarray(x)
    hidden = np.asarray(hidden)
    pool_x = np.asarray(pool_x)
    pool_h = np.asarray(pool_h)
    sw_x = np.asarray(sw_x)
    sw_h = np.asarray(sw_h)

    nc = _get_nc()
    in_maps = _prep_inputs(x, hidden, pool_x, pool_h, sw_x, sw_h)
    res = run_bass_kernel_spmd(
        nc, in_maps, core_ids=list(range(NCORES)), trace=_trace
    )
    if _results_holder is not None:
        _results_holder.append(res)

    out = np.empty((B, U, H), dtype=np.float32)
    for c in range(NCORES):
        hy_c = np.asarray(res.results[c]["hy"]).astype(np.float32)  # [UC, MC, 128, H]
        out[:, c * UC : (c + 1) * UC, :] = hy_c.reshape(UC, B, H).transpose(1, 0, 2)
    return out
